# revision 21
# baseline (speedup 1.0000x reference)
"""Trainium2 Bass kernel for the DSCNMP GNN (2x GINConv + pooling + MLP head).

Self-contained: takes full (unsharded) inputs, shards nodes/edges across the
8 NeuronCores internally, runs one SPMD Bass program via
bass_utils.run_bass_kernel_spmd, and returns the full [G, O] output.

Sharding strategy (per the problem's hint):
  - Nodes partitioned contiguously across 8 cores; each edge owned by the
    core of its dst node. Small MLP/BN weights replicated.
  - conv1 aggregation: edge stream of positions is fully host-precomputed
    (posE, bf16); per 128-edge slot a one-hot S (DVE is_equal) selects dst
    slots and the TensorEngine accumulates agg^T = land^T @ S in PSUM.
  - x1 is AllGathered per quarter (bf16, 4 quarter-tables) as soon as each
    quarter of x1 is computed, overlapping conv1 compute.
  - conv2 aggregation: per-edge x1 rows (256B) are fetched with
    gpsimd.dma_gather from the quarter tables.  Edges are grouped into
    (src-quarter, dst-window-512) cells so one PSUM bank [128,512]
    accumulates a whole window; this cuts the padded gather stream from
    ~106k to ~85k indices.  Chunk-major order lets chunk-0 gathers start
    right after the first AllGather; the chunk-3 pass finalizes each
    window (conv2 MLP, transpose, graph pool) so almost no tail remains.
  - Pooled graph embeddings AllReduced; graph-level head replicated.
"""

import numpy as np

N_FULL, E_FULL, G_FULL, C_DIM, H_DIM, O_DIM = 100000, 600000, 1000, 2, 128, 10
HC_DIM = H_DIM // 2
NCORES = 8
NCHUNK = 4          # int16 gather-index chunking of the global table
WWIN = 256          # conv2 dst-window width
WPG = 10            # windows per gather call
EPS = 1e-5

_CACHE = {}


def _pack_idx16(flat):
    """[j%16, j//16] int16 packing, replicated across the 8 Q7 groups."""
    total = len(flat)
    assert total % 16 == 0
    out = flat.reshape(total // 16, 16).T.astype(np.int16)
    return np.tile(out, (8, 1))


def _preprocess(pos, edge_index, batch, N, E, G):
    NL = N // NCORES
    NLP = -(-NL // 512) * 512
    NT = NLP // 128
    QL = NLP // NCHUNK
    NWIN = NLP // WWIN
    assert NWIN % WPG == 0
    assert QL * NCORES <= 32767 + 1

    pos = np.asarray(pos, np.float32)
    src = np.asarray(edge_index[0], np.int64)
    dst = np.asarray(edge_index[1], np.int64)
    batch = np.asarray(batch, np.int64)
    assert N % NCORES == 0

    node = np.arange(N)
    slot_of = NLP * (node // NL) + (node % NL)

    pos_nm = np.zeros((NCORES, 128, NT * C_DIM), np.float32)
    batch_rel = np.full((NCORES, 128, NT), -5.0, np.float32)
    g0 = np.zeros(NCORES, np.int64)
    gwin_need = 0
    for k in range(NCORES):
        nodes = np.arange(k * NL, (k + 1) * NL)
        j = nodes - k * NL
        pos_nm[k][j % 128, (j // 128) * C_DIM + 0] = pos[nodes, 0]
        pos_nm[k][j % 128, (j // 128) * C_DIM + 1] = pos[nodes, 1]
        g0[k] = batch[nodes[0]]
        rel = batch[nodes] - g0[k]
        batch_rel[k][j % 128, j // 128] = rel.astype(np.float32)
        gwin_need = max(gwin_need, int(rel.max()) + 1)
    GWIN = min(512, max(128, -(-gwin_need // 32) * 32))
    assert gwin_need <= GWIN <= 512
    WG = -(-(G + GWIN) // 256) * 256

    ecore = dst // NL
    ksrc = slot_of[src] // NLP
    jsrc = slot_of[src] % NLP
    grow_q = ksrc * QL + (jsrc % QL)     # row within quarter-table
    qsrc = jsrc // QL
    dslot = slot_of[dst] % NLP

    # ---- conv1 edge stream: cells = dst tile only, t-major ----
    tt_all = dslot // 128
    cells1 = [[None] * NT for _ in range(NCORES)]
    cnt1 = np.zeros((NCORES, NT), np.int64)
    for k in range(NCORES):
        m = ecore == k
        sk, dk = src[m], dslot[m]
        tk = dk // 128
        for t in range(NT):
            mm = tk == t
            dd = dk[mm]
            o = np.argsort(dd, kind="stable")
            cells1[k][t] = (sk[mm][o], dd[o] - t * 128)
            cnt1[k, t] = len(dd)
    slots1 = [-(-int(cnt1[:, t].max()) // 128) for t in range(NT)]
    off1 = np.concatenate([[0], np.cumsum(slots1)]).astype(np.int64)
    NSLOT1 = int(off1[-1])

    posE = np.zeros((NCORES, 128, NSLOT1 * C_DIM), np.float32)
    dwc1 = np.full((NCORES, 128, NSLOT1), -5.0, np.float32)
    for k in range(NCORES):
        pe = np.zeros((NSLOT1 * 128, C_DIM), np.float32)
        dw = np.full(NSLOT1 * 128, -5.0, np.float32)
        for t in range(NT):
            ss, dd = cells1[k][t]
            base = int(off1[t]) * 128
            pe[base:base + len(ss)] = pos[ss]
            dw[base:base + len(dd)] = dd.astype(np.float32)
        posE[k] = pe.reshape(NSLOT1, 128, C_DIM).transpose(1, 0, 2).reshape(
            128, NSLOT1 * C_DIM)
        dwc1[k] = dw.reshape(NSLOT1, 128).T

    # ---- conv2 edge stream: cells = (src quarter, dst window-512) ----
    ww_all = dslot // WWIN
    cells2 = [[[None] * NWIN for _ in range(NCHUNK)] for _ in range(NCORES)]
    cnt2 = np.zeros((NCORES, NCHUNK, NWIN), np.int64)
    for k in range(NCORES):
        m = ecore == k
        gq, ds_, ch, wk = grow_q[m], dslot[m], qsrc[m], ww_all[m]
        for c in range(NCHUNK):
            for w in range(NWIN):
                mm = (ch == c) & (wk == w)
                gg, dd = gq[mm], ds_[mm]
                o = np.argsort(dd, kind="stable")
                cells2[k][c][w] = (gg[o], dd[o] - w * WWIN)
                cnt2[k, c, w] = len(gg)
    slots2 = [[-(-int(cnt2[:, c, w].max()) // 128) for w in range(NWIN)]
              for c in range(NCHUNK)]
    off2 = np.concatenate(
        [[0], np.cumsum(np.array(slots2).reshape(-1))]).astype(np.int64)
    NSLOT2 = int(off2[-1])

    gidx2 = np.zeros((NCORES, 128, NSLOT2 * 8), np.int16)
    dwc2 = np.full((NCORES, 128, NSLOT2), -5, np.int16)
    for k in range(NCORES):
        gi = np.zeros(NSLOT2 * 128, np.int64)
        dw = np.full(NSLOT2 * 128, -5, np.int16)
        for c in range(NCHUNK):
            for w in range(NWIN):
                gg, dd = cells2[k][c][w]
                base = int(off2[c * NWIN + w]) * 128
                gi[base:base + len(gg)] = gg
                dw[base:base + len(dd)] = dd.astype(np.int16)
        gidx2[k] = _pack_idx16(gi)
        dwc2[k] = dw.reshape(NSLOT2, 128).T

    groff = np.zeros((NCORES, 1, 2), np.int32)
    groff[:, 0, 0] = g0
    assert (g0 + GWIN <= WG).all()

    dims = dict(N=N, E=E, G=G, NL=NL, NLP=NLP, NT=NT, QL=QL, NWIN=NWIN,
                NSLOT1=NSLOT1, NSLOT2=NSLOT2, GWIN=GWIN, WG=WG,
                slots1=tuple(slots1),
                slots2=tuple(tuple(r) for r in slots2))
    arrays = dict(posE=posE, pos_nm=pos_nm, batch_rel=batch_rel,
                  dwc1=dwc1, gidx2=gidx2, dwc2=dwc2, groff=groff)
    return dims, arrays


def _build_program(dims):
    import contextlib
    import concourse.bass as bass
    import concourse.bacc as bacc
    import concourse.mybir as mybir
    import concourse.tile as tile
    from concourse import library_config
    from concourse.masks import make_identity

    f32 = mybir.dt.float32
    bf16 = mybir.dt.bfloat16
    i16 = mybir.dt.int16
    i32 = mybir.dt.int32
    AF = mybir.ActivationFunctionType
    ALU = mybir.AluOpType

    NLP, NT, QL, NWIN = dims["NLP"], dims["NT"], dims["QL"], dims["NWIN"]
    NSLOT1, NSLOT2 = dims["NSLOT1"], dims["NSLOT2"]
    GWIN, WG, G = dims["GWIN"], dims["WG"], dims["G"]
    slots1 = list(dims["slots1"])
    slots2 = [list(r) for r in dims["slots2"]]
    off1 = np.concatenate([[0], np.cumsum(slots1)]).astype(np.int64)
    off2 = np.concatenate(
        [[0], np.cumsum(np.array(slots2).reshape(-1))]).astype(np.int64)
    QT = QL // 128                      # tiles per quarter (25)
    NWG = NWIN // WPG                   # gather groups per chunk
    MAXSL1 = max(slots1)
    MAXCALL = max(sum(slots2[c][wg * WPG:(wg + 1) * WPG])
                  for c in range(NCHUNK) for wg in range(NWG))
    MAXSL2 = max(max(r) for r in slots2)
    # ragged MLP windows, aligned to quarters: 6x512 + 1x128 per quarter
    wins = []
    for q in range(4):
        c0 = q * QL
        while c0 < (q + 1) * QL:
            cw = min(512, (q + 1) * QL - c0)
            wins.append((c0, cw))
            c0 += cw

    nc = bacc.Bacc("TRN2", target_bir_lowering=False, debug=False,
                   enable_asserts=True, num_devices=NCORES)

    def din(name, shape, dt=f32):
        return nc.dram_tensor(name, list(shape), dt, kind="ExternalInput")

    posE_d = din("posE", [128, NSLOT1 * C_DIM], bf16)
    pos_nm_d = din("pos_nm", [128, NT * C_DIM], bf16)
    batch_rel_d = din("batch_rel", [128, NT])
    dwc1_d = din("dwc1", [128, NSLOT1])
    gidx2_d = din("gidx2", [128, NSLOT2 * 8], i16)
    dwc2_d = din("dwc2", [128, NSLOT2], i16)
    groff_d = din("groff", [1, 2], i32)
    iota_d = din("iota", [128, 512])
    iota16_d = din("iota16", [128, 512], i16)

    wnames = {}
    for nm, shp in [("W1a", [C_DIM, H_DIM]), ("W1b", [H_DIM, H_DIM]),
                    ("W2a", [H_DIM, H_DIM]), ("W2b", [H_DIM, H_DIM]),
                    ("Wf1", [C_DIM, H_DIM]), ("Wf2", [H_DIM, H_DIM]),
                    ("Wc1", [H_DIM, HC_DIM]), ("Wc2", [HC_DIM, O_DIM])]:
        wnames[nm] = din(nm, shp)
    vecs = {}
    for nm in ["b1a", "b1b", "b2a", "b2b", "bf1", "bf2",
               "n1_g", "n1_b", "n1_rm", "n1_rv", "n2_g", "n2_b", "n2_rm", "n2_rv",
               "f1_g", "f1_b", "f1_rm", "f1_rv", "f2_g", "f2_b", "f2_rm", "f2_rv"]:
        vecs[nm] = din(nm, [H_DIM, 1])
    for nm in ["bc1", "gc", "bec", "rmc", "rvc", "a_prelu_v"]:
        vecs[nm] = din(nm, [HC_DIM, 1])
    vecs["bc2"] = din("bc2", [O_DIM, 1])

    out_d = nc.dram_tensor("out", [G, O_DIM], f32, kind="ExternalOutput")

    with tile.TileContext(nc) as tc:
        nc.gpsimd.load_library(library_config.mlp)
        ctx = contextlib.ExitStack()
        with ctx:
            dram = ctx.enter_context(tc.tile_pool(name="dram", bufs=1, space="DRAM"))
            pconst = ctx.enter_context(tc.tile_pool(name="const", bufs=1))
            pbig = ctx.enter_context(tc.tile_pool(name="big", bufs=1))
            pland = ctx.enter_context(tc.tile_pool(name="land", bufs=2))
            psmall = ctx.enter_context(tc.tile_pool(name="small", bufs=2))
            ps1 = ctx.enter_context(tc.tile_pool(name="s1", bufs=2))
            pgr = ctx.enter_context(tc.tile_pool(name="gr", bufs=1))
            ph1 = ctx.enter_context(tc.tile_pool(name="h1w", bufs=2))
            ppsum = ctx.enter_context(tc.tile_pool(name="psum", bufs=2, space="PSUM"))
            pseg = ctx.enter_context(tc.tile_pool(name="psum_seg", bufs=2, space="PSUM"))
            ppool = ctx.enter_context(tc.tile_pool(name="psum_acc", bufs=2, space="PSUM"))

            cc_in = [dram.tile([QL, H_DIM], bf16, tag="cc_in", name=f"cc_in{q}",
                                bufs=NCHUNK) for q in range(NCHUNK)]
            cc_out = [dram.tile([QL * NCORES, H_DIM], bf16, tag="cc_out",
                                name=f"cc_out{q}", addr_space="Shared",
                                bufs=NCHUNK) for q in range(NCHUNK)]
            ar_in = dram.tile([2 * H_DIM + C_DIM, WG], f32, tag="ar_in")
            ar_out = dram.tile([2 * H_DIM + C_DIM, WG], f32, tag="ar_out",
                               addr_space="Shared")

            def load_const(dr, shape, dt=f32):
                t = pconst.tile(shape, dt, tag=dr.name + "_sb")
                nc.sync.dma_start(out=t[:], in_=dr.ap())
                return t

            W = {k: load_const(v, v.shape) for k, v in wnames.items()}
            V = {k: load_const(v, v.shape) for k, v in vecs.items()}
            pos_nm = load_const(pos_nm_d, [128, NT * C_DIM], bf16)
            posE = load_const(posE_d, [128, NSLOT1 * C_DIM], bf16)
            batch_rel = load_const(batch_rel_d, [128, NT])
            iota = load_const(iota_d, [128, 512])
            dwc1 = load_const(dwc1_d, [128, NSLOT1])
            gidx2 = load_const(gidx2_d, [128, NSLOT2 * 8], i16)
            dwc2 = load_const(dwc2_d, [128, NSLOT2], i16)
            iota16 = load_const(iota16_d, [128, 512], i16)
            groff = load_const(groff_d, [1, 2], i32)

            ident = pconst.tile([128, 128], f32, tag="ident")
            make_identity(nc, ident[:])
            iota_bf = pconst.tile([128, 512], bf16, tag="iota_bf")
            nc.vector.tensor_copy(iota_bf[:], iota[:])
            ident_bf = pconst.tile([128, 128], bf16, tag="ident_bf")
            nc.vector.tensor_copy(ident_bf[:], ident[:])

            def bn_vec(g, b, rm, rv, P, nm):
                a = pconst.tile([P, 1], f32, tag=f"bn_a_{nm}")
                c = pconst.tile([P, 1], f32, tag=f"bn_c_{nm}")
                nc.vector.tensor_scalar(a[:], rv[:], EPS, None, ALU.add)
                nc.scalar.activation(a[:], a[:], AF.Sqrt)
                nc.vector.reciprocal(a[:], a[:])
                nc.vector.tensor_tensor(a[:], a[:], g[:], op=ALU.mult)
                nc.vector.tensor_tensor(c[:], rm[:], a[:], op=ALU.mult)
                nc.vector.tensor_tensor(c[:], b[:], c[:], op=ALU.subtract)
                return a, c
            a1, c1 = bn_vec(V["n1_g"], V["n1_b"], V["n1_rm"], V["n1_rv"], H_DIM, "n1")
            a2, c2 = bn_vec(V["n2_g"], V["n2_b"], V["n2_rm"], V["n2_rv"], H_DIM, "n2")
            af1, cf1 = bn_vec(V["f1_g"], V["f1_b"], V["f1_rm"], V["f1_rv"], H_DIM, "f1")
            af2, cf2 = bn_vec(V["f2_g"], V["f2_b"], V["f2_rm"], V["f2_rv"], H_DIM, "f2")
            acl, ccl = bn_vec(V["gc"], V["bec"], V["rmc"], V["rvc"], HC_DIM, "cls")

            W1a_bf = pconst.tile([C_DIM, H_DIM], bf16, tag="W1a_bf")
            nc.vector.tensor_copy(W1a_bf[:], W["W1a"][:])
            W1b_bf = pconst.tile([H_DIM, H_DIM], bf16, tag="W1b_bf")
            nc.vector.tensor_copy(W1b_bf[:], W["W1b"][:])
            W2b_bf = pconst.tile([H_DIM, H_DIM], bf16, tag="W2b_bf")
            nc.vector.tensor_copy(W2b_bf[:], W["W2b"][:])

            # persistent big buffers
            zbuf = pbig.tile([128, NLP], bf16, tag="A")      # z1 then z2 (bf16)
            xT = pbig.tile([128, NLP], f32, tag="B")         # x1T -> h2T -> x2T
            xnm = pbig.tile([128, NT * H_DIM], bf16, tag="NM")  # x1nm then x2nm

            posE_v = posE[:].rearrange("p (s c) -> p s c", c=C_DIM)
            h1w = {}

            # =============== phase 1: conv1, per quarter ===============
            for q in range(4):
                for t in range(q * QT, (q + 1) * QT):
                    ns = slots1[t]
                    s0 = int(off1[t])
                    ps = pseg.tile([C_DIM, 128], f32, tag="seg1", bufs=1)
                    if ns:
                        S8 = ps1.tile([128, MAXSL1 * 128], bf16, tag="S8")
                        nc.vector.tensor_tensor(
                            out=S8[:, 0:ns * 128].rearrange(
                                "p (s j) -> p s j", j=128),
                            in0=iota_bf[:, None, 0:128].to_broadcast(
                                [128, ns, 128]),
                            in1=dwc1[:, s0:s0 + ns, None].to_broadcast(
                                [128, ns, 128]),
                            op=ALU.is_equal)
                        for sl in range(ns):
                            nc.tensor.matmul(ps[:], posE_v[:, s0 + sl, :],
                                             S8[:, sl * 128:(sl + 1) * 128],
                                             start=(sl == 0), stop=False)
                    nc.tensor.matmul(ps[:], pos_nm[:, t * C_DIM:(t + 1) * C_DIM],
                                     ident_bf[:], start=(ns == 0), stop=True)
                    # flush into ragged h1 window (bf16)
                    o = (t - q * QT) * 128
                    wloc = o // 512
                    wi = q * 7 + wloc
                    if wi not in h1w:
                        h1w[wi] = ph1.tile([C_DIM, wins[wi][1]], bf16,
                                           tag=f"h1w{wins[wi][1]}", name=f"h1w{wi}")
                    nc.scalar.copy(h1w[wi][:, o - wloc * 512:o - wloc * 512 + 128],
                                   ps[0:C_DIM, :])
                # conv1 MLP for this quarter's windows
                for wi in range(q * 7, (q + 1) * 7):
                    c0, cw = wins[wi]
                    psm = ppsum.tile([H_DIM, 512], f32, tag="work")
                    nc.tensor.matmul(psm[:, 0:cw], W1a_bf[:], h1w[wi][:],
                                     start=True, stop=True)
                    nc.scalar.activation(zbuf[:, c0:c0 + cw], psm[:, 0:cw],
                                         AF.Relu, bias=V["b1a"][:], scale=1.0)
                    psm2 = ppsum.tile([H_DIM, 512], f32, tag="work")
                    nc.tensor.matmul(psm2[:, 0:cw], W1b_bf[:], zbuf[:, c0:c0 + cw],
                                     start=True, stop=True)
                    nc.scalar.activation(psm2[:, 0:cw], psm2[:, 0:cw], AF.Relu,
                                         bias=V["b1b"][:], scale=1.0)
                    nc.vector.tensor_scalar(xT[:, c0:c0 + cw], psm2[:, 0:cw],
                                            a1[:], c1[:], ALU.mult, ALU.add)
                # transpose to node-major + AllGather this quarter
                for t in range(q * QT, (q + 1) * QT):
                    pt = ppsum.tile([128, 128], f32, tag="work")
                    nc.tensor.transpose(pt[:], xT[:, t * 128:(t + 1) * 128], ident[:])
                    nc.scalar.copy(xnm[:, t * 128:(t + 1) * 128], pt[:])
                nc.sync.dma_start(
                    out=cc_in[q][:].rearrange("(s p) f -> p s f", p=128),
                    in_=xnm[:, q * QT * H_DIM:(q + 1) * QT * H_DIM].rearrange(
                        "p (s f) -> p s f", f=H_DIM))
                with tc.high_priority():
                    nc.gpsimd.collective_compute(
                        "AllGather", mybir.AluOpType.bypass,
                        ins=[cc_in[q].opt()], outs=[cc_out[q].opt()],
                        replica_groups=[list(range(NCORES))])

            # =============== conv2: gather + window seg-sum ===============
            # all three pools (pos, x1, x2) are accumulated during the
            # finalize pass; one merged AllReduce at the end.
            ps_pos = ppool.tile([C_DIM, GWIN], f32, tag="accp", bufs=1)
            ps_x1 = ppool.tile([128, GWIN], f32, tag="acc")
            ps_x2 = ppool.tile([128, GWIN], f32, tag="acc")
            tglob = [0]

            def finalize_block(wb):
                # h2 columns [512*wb, 512*(wb+1)) complete -> conv2 MLP ->
                # x2 -> pools of pos/x1/x2 (one B build per tile)
                c0 = wb * 512
                psm = ppsum.tile([H_DIM, 512], f32, tag="work")
                nc.tensor.matmul(psm[:], W["W2a"][:], xT[:, c0:c0 + 512],
                                 start=True, stop=True)
                nc.scalar.activation(zbuf[:, c0:c0 + 512], psm[:],
                                     AF.Relu, bias=V["b2a"][:], scale=1.0)
                psm2 = ppsum.tile([H_DIM, 512], f32, tag="work")
                nc.tensor.matmul(psm2[:], W2b_bf[:], zbuf[:, c0:c0 + 512],
                                 start=True, stop=True)
                nc.scalar.activation(psm2[:], psm2[:], AF.Relu,
                                     bias=V["b2b"][:], scale=1.0)
                nc.vector.tensor_scalar(xT[:, c0:c0 + 512], psm2[:],
                                        a2[:], c2[:], ALU.mult, ALU.add)
                for t in range(wb * 4, wb * 4 + 4):
                    tg = tglob[0]
                    B = psmall.tile([128, GWIN], bf16, tag="B")
                    nc.vector.tensor_scalar(B[:], iota_bf[:, 0:GWIN],
                                            batch_rel[:, t:t + 1], None,
                                            ALU.is_equal)
                    nc.tensor.matmul(ps_pos[:],
                                     pos_nm[:, t * C_DIM:(t + 1) * C_DIM],
                                     B[:], start=(tg == 0), stop=(tg == NT - 1))
                    nc.tensor.matmul(ps_x1[:], xnm[:, t * 128:(t + 1) * 128],
                                     B[:], start=(tg == 0), stop=(tg == NT - 1))
                    pt = ppsum.tile([128, 128], f32, tag="work")
                    nc.tensor.transpose(pt[:], xT[:, t * 128:(t + 1) * 128],
                                        ident[:])
                    nc.scalar.copy(xnm[:, t * 128:(t + 1) * 128], pt[:])
                    nc.tensor.matmul(ps_x2[:], xnm[:, t * 128:(t + 1) * 128],
                                     B[:], start=(tg == 0), stop=(tg == NT - 1))
                    tglob[0] += 1

            for c in range(NCHUNK):
                for wg in range(NWG):
                    w0 = wg * WPG
                    sbase = int(off2[c * NWIN + w0])
                    nsl_call = sum(slots2[c][w0:w0 + WPG])
                    land = pland.tile([128, MAXCALL, H_DIM], bf16, tag="land")
                    nc.gpsimd.dma_gather(
                        land[:, 0:nsl_call, :], cc_out[c][:],
                        gidx2[:, sbase * 8:(sbase + nsl_call) * 8],
                        nsl_call * 128, nsl_call * 128, H_DIM,
                        single_packet=False)
                    loff = 0
                    for w in range(w0, w0 + WPG):
                        ns = slots2[c][w]
                        s0 = int(off2[c * NWIN + w])
                        if ns == 0:
                            if c == NCHUNK - 1 and w % 2 == 1:
                                finalize_block(w // 2)
                            continue
                        S = psmall.tile([128, MAXSL2 * WWIN], bf16, tag="S2")
                        nc.vector.tensor_tensor(
                            out=S[:, 0:ns * WWIN].rearrange(
                                "p (s j) -> p s j", j=WWIN),
                            in0=iota16[:, None, 0:WWIN].to_broadcast(
                                [128, ns, WWIN]),
                            in1=dwc2[:, s0:s0 + ns, None].to_broadcast(
                                [128, ns, WWIN]),
                            op=ALU.is_equal)
                        ps = pseg.tile([128, WWIN], f32, tag="seg2")
                        for sl in range(ns):
                            nc.tensor.matmul(ps[:], land[:, loff + sl, :],
                                             S[:, sl * WWIN:(sl + 1) * WWIN],
                                             start=(sl == 0), stop=(sl == ns - 1))
                        loff += ns
                        cols = slice(w * WWIN, (w + 1) * WWIN)
                        nc.vector.tensor_tensor(xT[:, cols], xT[:, cols], ps[:],
                                                op=ALU.add)
                        if c == NCHUNK - 1 and w % 2 == 1:
                            finalize_block(w // 2)

            # merged pools -> one AllReduce of [x1; x2; pos] rows
            arin_x1 = pgr.tile([H_DIM, GWIN], f32, tag="arin", bufs=2)
            nc.scalar.copy(arin_x1[:], ps_x1[:])
            arin_x2 = pgr.tile([H_DIM, GWIN], f32, tag="arin", bufs=2)
            nc.scalar.copy(arin_x2[:], ps_x2[:])
            arin_pos = pgr.tile([C_DIM, GWIN], f32, tag="arin_p")
            nc.scalar.copy(arin_pos[:], ps_pos[:])
            zrow = pgr.tile([H_DIM, 256], f32, tag="zrow")
            nc.vector.memset(zrow[:], 0.0)
            for zc in range(0, WG, 256):
                nc.sync.dma_start(out=ar_in[0:H_DIM, zc:zc + 256], in_=zrow[:])
                nc.sync.dma_start(out=ar_in[H_DIM:2 * H_DIM, zc:zc + 256],
                                  in_=zrow[:])
                nc.sync.dma_start(out=ar_in[2 * H_DIM:, zc:zc + 256],
                                  in_=zrow[0:C_DIM, :])
            with nc.gpsimd.register("g0r") as g0r:
                nc.gpsimd.reg_load(g0r, groff[0:1, 0:1])
                sv = nc.gpsimd.snap(g0r, min_val=0, max_val=WG - GWIN)
            nc.gpsimd.dma_start(out=ar_in[0:H_DIM, bass.ds(sv, GWIN)],
                                in_=arin_x1[:])
            nc.gpsimd.dma_start(out=ar_in[H_DIM:2 * H_DIM, bass.ds(sv, GWIN)],
                                in_=arin_x2[:])
            nc.gpsimd.dma_start(out=ar_in[2 * H_DIM:, bass.ds(sv, GWIN)],
                                in_=arin_pos[:])
            nc.gpsimd.collective_compute(
                "AllReduce", mybir.AluOpType.add,
                ins=[ar_in.opt()], outs=[ar_out.opt()],
                replica_groups=[list(range(NCORES))])

            # =============== graph stage ===============
            def g_mlp(lhsT_w, rhs, out, bias, bn, P=H_DIM, relu=True):
                for w in range(-(-WG // 512)):
                    c0 = w * 512
                    cw = min(512, WG - c0)
                    ps = ppsum.tile([P, 512], f32, tag="work")
                    nc.tensor.matmul(ps[:P, :cw], lhsT_w[:], rhs[:, c0:c0 + cw],
                                     start=True, stop=True)
                    fn = AF.Relu if relu else AF.Identity
                    nc.scalar.activation(ps[:P, :cw], ps[:P, :cw], fn,
                                         bias=bias[:], scale=1.0)
                    if bn is not None:
                        a_, c_ = bn
                        nc.vector.tensor_scalar(out[:, c0:c0 + cw], ps[:P, :cw],
                                                a_[:], c_[:], ALU.mult, ALU.add)
                    else:
                        nc.scalar.copy(out[:, c0:c0 + cw], ps[:P, :cw])

            ar1x = pgr.tile([H_DIM, WG], f32, tag="arbig", bufs=2)
            nc.sync.dma_start(out=ar1x[:], in_=ar_out[0:H_DIM, :])
            ar2x = pgr.tile([H_DIM, WG], f32, tag="arbig", bufs=2)
            nc.sync.dma_start(out=ar2x[:], in_=ar_out[H_DIM:2 * H_DIM, :])
            ar1p = pgr.tile([C_DIM, WG], f32, tag="ar1p")
            nc.sync.dma_start(out=ar1p[:], in_=ar_out[2 * H_DIM:, :])

            x0g = pgr.tile([H_DIM, WG], f32, tag="g_x0g")
            g_mlp(W["Wf1"], ar1p[:], x0g, V["bf1"], (af1, cf1))
            tmp = pgr.tile([H_DIM, WG], f32, tag="g_tmp")
            nc.vector.tensor_tensor(tmp[:], x0g[:], ar1x[:], op=ALU.add)
            x1g = pgr.tile([H_DIM, WG], f32, tag="g_x1g", bufs=2)
            g_mlp(W["Wf2"], tmp, x1g, V["bf2"], (af2, cf2))
            nc.vector.tensor_tensor(tmp[:], x0g[:], x1g[:], op=ALU.add)
            nc.vector.tensor_tensor(tmp[:], tmp[:], ar2x[:], op=ALU.add)
            x2g = pgr.tile([H_DIM, WG], f32, tag="g_x0g")
            g_mlp(W["Wf2"], tmp, x2g, V["bf2"], (af2, cf2))

            hcls = pgr.tile([HC_DIM, WG], f32, tag="g_tmp")
            g_mlp(W["Wc1"], x2g, hcls, V["bc1"], (acl, ccl), P=HC_DIM, relu=False)
            hneg = pgr.tile([HC_DIM, WG], f32, tag="g_x1g", bufs=2)
            nc.vector.tensor_scalar(hneg[:], hcls[:], V["a_prelu_v"][:], None,
                                    ALU.mult)
            nc.vector.tensor_tensor(hcls[:], hcls[:], hneg[:], op=ALU.max)
            outT = pgr.tile([O_DIM, WG], f32, tag="g_x1g", bufs=2)
            g_mlp(W["Wc2"], hcls, outT, V["bc2"], None, P=O_DIM, relu=False)

            ngt = -(-G // 128)
            onm = pgr.tile([128, ngt * O_DIM], f32, tag="onm")
            for j in range(ngt):
                pt = ppsum.tile([128, 128], f32, tag="work")
                nc.tensor.transpose(pt[:, 0:O_DIM], outT[:, j * 128:(j + 1) * 128],
                                    ident[0:O_DIM, 0:O_DIM])
                nc.scalar.copy(onm[:, j * O_DIM:(j + 1) * O_DIM], pt[:, 0:O_DIM])
            nfull = G // 128
            if nfull:
                nc.sync.dma_start(
                    out=out_d.ap()[0:nfull * 128, :].rearrange(
                        "(s p) o -> p s o", p=128),
                    in_=onm[:, :nfull * O_DIM].rearrange(
                        "p (s o) -> p s o", o=O_DIM))
            rem = G - nfull * 128
            if rem:
                nc.sync.dma_start(out=out_d.ap()[nfull * 128:G, :],
                                  in_=onm[0:rem, nfull * O_DIM:(nfull + 1) * O_DIM])

    nc.compile()
    return nc


def _build_in_maps(inputs, dims, arrays):
    import ml_dtypes
    f = lambda x: np.ascontiguousarray(np.asarray(x, np.float32))
    col = lambda x: f(x).reshape(-1, 1)
    shared = {
        "iota": np.tile(np.arange(512, dtype=np.float32), (128, 1)),
        "iota16": np.tile(np.arange(512, dtype=np.int16), (128, 1)),
        "W1a": f(inputs["W1a"]), "W1b": f(inputs["W1b"]),
        "W2a": f(inputs["W2a"]), "W2b": f(inputs["W2b"]),
        "Wf1": f(inputs["Wf1"]), "Wf2": f(inputs["Wf2"]),
        "Wc1": f(inputs["Wc1"]), "Wc2": f(inputs["Wc2"]),
        "b1a": col(inputs["b1a"]), "b1b": col(inputs["b1b"]),
        "b2a": col(inputs["b2a"]), "b2b": col(inputs["b2b"]),
        "bf1": col(inputs["bf1"]), "bf2": col(inputs["bf2"]),
        "bc1": col(inputs["bc1"]), "bc2": col(inputs["bc2"]),
        "gc": col(inputs["gc"]), "bec": col(inputs["bec"]),
        "rmc": col(inputs["rmc"]), "rvc": col(inputs["rvc"]),
        "a_prelu_v": np.full((HC_DIM, 1),
                             np.float32(np.asarray(inputs["a_prelu"]))),
    }
    for pfx in ["n1_", "n2_", "f1_", "f2_"]:
        for sfx in ["g", "b", "rm", "rv"]:
            shared[pfx + sfx] = col(inputs[pfx + sfx])
    in_maps = []
    for k in range(NCORES):
        m = dict(shared)
        m["posE"] = arrays["posE"][k].astype(ml_dtypes.bfloat16)
        m["pos_nm"] = arrays["pos_nm"][k].astype(ml_dtypes.bfloat16)
        m["batch_rel"] = arrays["batch_rel"][k]
        m["dwc1"] = arrays["dwc1"][k]
        m["gidx2"] = arrays["gidx2"][k]
        m["dwc2"] = arrays["dwc2"][k]
        m["groff"] = arrays["groff"][k]
        in_maps.append(m)
    return in_maps


def _get_compiled(pos, edge_index, batch, N, E, G):
    dims, arrays = _preprocess(pos, edge_index, batch, N, E, G)
    key = tuple(sorted((k, str(v)) for k, v in dims.items()))
    if key not in _CACHE:
        _CACHE[key] = _build_program(dims)
    return _CACHE[key], dims, arrays


def kernel(**inputs):
    from concourse.bass_utils import run_bass_kernel_spmd
    pos = np.asarray(inputs["pos"])
    ei = np.asarray(inputs["edge_index"])
    batch = np.asarray(inputs["batch"])
    nc, dims, arrays = _get_compiled(pos, ei, batch, pos.shape[0],
                                     ei.shape[1], G_FULL)
    in_maps = _build_in_maps(inputs, dims, arrays)
    res = run_bass_kernel_spmd(nc, in_maps, list(range(NCORES)))
    return np.asarray(res.results[0]["out"], np.float32)
